# revision 14
# baseline (speedup 1.0000x reference)
"""Trainium2 Bass kernel for DiscriminatorAugment (translation + color jitter +
cutout), data-parallel over 8 NeuronCores (8 samples each).

Math: with x0 = translated image, the reference's color jitter chain
    x1 = x0 + badd;  x2 = (x1 - mean_c x1)*s + mean_c x1;
    x3 = (x2 - mean_chw x2)*t + mean_chw x2
collapses to the per-pixel affine
    x3 = A*x0 + BC*m3 + C,   A = t*s, BC = t*(1-s)/3, m3 = sum_c x0,
    C = (1-t)*g0 + badd,     g0 = (sum_chw x0)/(3*H*W)
and cutout multiplies by (1 - rowmask*colmask).

DMA layout: the per-core ceiling is ~365 GB/s (16 shared DMA engines) and is
descriptor-size independent, but small descriptors inflate desc-gen latency
and stall the engines behind compute semaphores. So every bulk transfer is
one contiguous 8KB descriptor per partition:
  - input is padded vertically only ([C, 642, 512]: 1 guard row, 64 pad rows,
    image, 64 pad rows, 1 guard row; rows stay contiguous). BOTH translation
    offsets fold into a single dynamic element offset (65+th)*512 + tw on the
    load: partition p receives out-rows 4p..4p+3 as one flat 8KB run. Reads
    that cross a row boundary (column spill) land in the next DRAM row's
    leading columns; those out-positions are exactly the reference's
    zero-padding region, so they are zeroed right after the load by a
    per-sample 0/1 edge-mask multiply on two static 64-wide column windows
    (host supplies the vectors). After that every downstream op is a static
    AP and reproduces the reference exactly, including the global mean.
  - stores go from compact per-channel tiles to the contiguous output,
    8KB/partition.
  - the cutout mask (bf16 in DRAM, halved traffic) is cast-loaded to f32 by
    SWDGE.
Engine roles avoid head-of-line blocking of DMA issue behind compute waits:
sync+scalar alternate issuing each sample's loads (HWDGE), gpsimd issues
stores + mask loads (SWDGE), DVE does the elementwise work (cutout multiply
alternates DVE/GpSimd), PE the cross-partition sum, ACT only the [P,1] C.
Dynamic values: one load offset per sample on the issuing engine, one cutout
start on the multiply engine - 4 per engine, under the ~7-value register cap.
"""
import threading

import numpy as np
import ml_dtypes

import concourse.bass as bass
import concourse.mybir as mybir
import concourse.tile as tile
from concourse.bass_utils import run_bass_kernel_spmd

M = 8          # cores
B = 64         # full batch
BS = B // M    # samples per core
C, H, W = 3, 512, 512
PAD = 64       # translation margin (delta_h = delta_w = 64)
HV = H + 2 * PAD + 2   # padded rows per channel: guard + 64 + 512 + 64 + guard
P = 128
NJ = H // P    # 4 consecutive rows per partition
CH = round(H * 0.2)   # 102 cutout rows
CW = 104              # static cutout column window (covers any clipped range)
F32 = mybir.dt.float32
BF16 = mybir.dt.bfloat16
I32 = mybir.dt.int32

# pf columns
I_A, I_BC, I_GS, I_BADD = 0, 1, 2, 3

XBUFS = 5


def _split_waits(nc, max_waits=1):
    """Walrus in this container rejects >2 sem waits on one instruction
    ("Too many sync wait commands"). Hoist excess waits onto standalone
    single-wait event-semaphore instructions immediately before, same
    engine — semantics identical (waits execute before the instruction
    in program order either way)."""
    uid = 0
    for f in nc.m.functions:
        for bb in f.blocks:
            new_list, changed = [], False
            for inst in bb.instructions:
                si = inst.sync_info
                waits = list(si.on_wait) if si and si.on_wait else []
                if len(waits) > max_waits:
                    changed = True
                    for w in waits[:-max_waits]:
                        uid += 1
                        ev = mybir.InstEventSemaphore(name=f"splitwait_{uid}")
                        ev.engine = inst.engine
                        ev.sync_info = mybir.SyncInfo(on_wait=[w], on_update=[])
                        new_list.append(ev)
                    inst.sync_info = mybir.SyncInfo(
                        on_wait=waits[-max_waits:],
                        on_update=list(si.on_update) if si.on_update else [],
                    )
                new_list.append(inst)
            if changed:
                bb.instructions = new_list


def _bcast_part(ap, p=P):
    """Replicate a DRAM AP across p partitions (0-stride partition dim)."""
    return bass.AP(tensor=ap.tensor, offset=ap.offset, ap=[[0, p]] + list(ap.ap))


def _build_program():
    nc = bass.Bass(num_swdge_queues=1)
    img = nc.declare_dram_parameter("img", [BS, C, HV, W], F32, isOutput=False)
    pf = nc.declare_dram_parameter("pf", [BS, 4], F32, isOutput=False)
    pi = nc.declare_dram_parameter("pi", [BS, 1], I32, isOutput=False)
    pcs = nc.declare_dram_parameter("pcs", [BS, 1], I32, isOutput=False)
    el = nc.declare_dram_parameter("el", [BS, 2, PAD], F32, isOutput=False)
    invw = nc.declare_dram_parameter("invw", [BS, P, NJ, CW], BF16, isOutput=False)
    out = nc.declare_dram_parameter("out", [BS, C, H, W], F32, isOutput=True)

    Alu = mybir.AluOpType
    Act = mybir.ActivationFunctionType
    SP = mybir.EngineType.SP
    ACT = mybir.EngineType.Activation
    DVE = mybir.EngineType.DVE
    POOL = mybir.EngineType.Pool

    with tile.TileContext(nc) as tc:
        with (
            tc.tile_pool(name="xp", bufs=XBUFS) as xp,
            tc.tile_pool(name="mp", bufs=2) as mp,
            tc.tile_pool(name="op", bufs=2) as op,
            tc.tile_pool(name="wp", bufs=3) as wp,
            tc.tile_pool(name="singles", bufs=1) as singles,
            tc.tile_pool(name="psum", bufs=4, space="PSUM") as psum,
        ):
            ones_t = singles.tile([P, P], F32)
            nc.vector.memset(ones_t[:], 1.0)
            pf_sb = singles.tile([P, BS, 4], F32)
            nc.scalar.dma_start(out=pf_sb[:], in_=_bcast_part(pf[:]))
            el_sb = singles.tile([P, BS, 2, PAD], F32)
            nc.scalar.dma_start(out=el_sb[:], in_=_bcast_part(el[:]))
            # stage the dynamic offsets in SBUF: register loads from DRAM
            # take ~2-3us on the issuing engine, from SBUF they are cheap
            pi_sb = singles.tile([1, BS], I32)
            nc.sync.dma_start(out=pi_sb[:], in_=pi[:].rearrange("b one -> one b"))
            pcs_sb = singles.tile([1, BS], I32)
            nc.scalar.dma_start(out=pcs_sb[:], in_=pcs[:].rearrange("b one -> one b"))
            # all 8 cutout masks up front on HWDGE, kept bf16 (the multiply
            # reads bf16 directly) - keeps SWDGE ring0 free for stores
            invw_sb = singles.tile([P, BS, NJ, CW], BF16)
            nc.sync.dma_start(
                out=invw_sb[:], in_=invw[:].rearrange("b p j w -> p b j w")
            )

            state = {}

            def stage_load(b):
                ld_eng = nc.sync if b % 2 == 0 else nc.scalar
                ld_eng_t = SP if b % 2 == 0 else ACT
                x_t = xp.tile([P, C, NJ, W], F32, tag="x")
                off = nc.values_load(
                    pi_sb[0:1, b : b + 1],
                    engines=[ld_eng_t],
                    min_val=W - PAD,
                    max_val=(1 + 2 * PAD) * W + PAD,
                    skip_runtime_bounds_check=True,
                )
                # one DMA for all 3 channels: partition p gets, per channel,
                # out-rows 4p..4p+3 as one flat 8KB run (register budget:
                # each dynamic-AP instruction costs an engine register slot)
                base = img[b, 0]
                src = bass.AP(
                    tensor=base.tensor,
                    offset=base.offset + off,
                    ap=[[NJ * W, P], [HV * W, C], [W, NJ], [1, W]],
                )
                ld_eng.dma_start(out=x_t[:], in_=src)
                state[b] = dict(x_t=x_t)

            def stage_m3(b):
                st = state[b]
                x_t = st["x_t"]
                # zero the columns where the flat shifted load spilled across
                # a row boundary == the reference's translation zero-padding:
                # head cols [0,64) when tw<0, tail cols [448,512) when tw>0
                base = x_t[:]
                win = bass.AP(
                    tensor=base.tensor,
                    offset=base.offset,
                    ap=[
                        list(base.ap[0]),
                        [NJ * W, C],
                        [W, NJ],
                        [W - PAD, 2],
                        [1, PAD],
                    ],
                )
                elm = el_sb[:, b : b + 1, None, :, :].broadcast_to(
                    [P, C, NJ, 2, PAD]
                )
                nc.vector.tensor_tensor(win, win, elm, Alu.mult)
                # m3 = c0+c1+c2, fused row-sum into s_t
                m3_t = mp.tile([P, NJ, W], F32, tag="m3")
                s_t = wp.tile([P, 1], F32, tag="s")
                c_t = wp.tile([P, 1], F32, tag="c")
                g_t = psum.tile([P, 1], F32, tag="g")
                nc.vector.tensor_tensor(m3_t[:], x_t[:, 0], x_t[:, 1], Alu.add)
                nc.vector.scalar_tensor_tensor(
                    out=m3_t[:],
                    in0=m3_t[:],
                    scalar=0.0,
                    in1=x_t[:, 2],
                    op0=Alu.bypass,
                    op1=Alu.add,
                    accum_out=s_t[:],
                )
                # cross-partition sum, broadcast to all partitions
                nc.tensor.matmul(g_t[:], ones_t[:], s_t[:], start=True, stop=True)
                # C = GS * total + badd   (per-partition [P,1])
                nc.scalar.activation(
                    c_t[:],
                    g_t[:],
                    Act.Identity,
                    bias=pf_sb[:, b, I_BADD : I_BADD + 1],
                    scale=pf_sb[:, b, I_GS : I_GS + 1],
                )
                # D = BC*m3 + C  (in place over m3, on ACT: per-partition
                # scale+bias, keeps DVE free)
                nc.scalar.activation(
                    m3_t[:],
                    m3_t[:],
                    Act.Identity,
                    bias=c_t[:],
                    scale=pf_sb[:, b, I_BC : I_BC + 1],
                )
                st["m3_t"] = m3_t

            def stage_out(b):
                st = state.pop(b)
                x_t, m3_t = st["x_t"], st["m3_t"]
                cut_eng = nc.vector if b % 4 != 3 else nc.gpsimd
                cs = nc.values_load(
                    pcs_sb[0:1, b : b + 1],
                    engines=[DVE if b % 4 != 3 else POOL],
                    min_val=0,
                    max_val=W - CW,
                    skip_runtime_bounds_check=True,
                )
                o_t = op.tile([P, C, NJ, W], F32, tag="o")
                # the last samples are the pipeline drain: go per-channel so
                # channel 0's store starts before channel 2 is computed
                perch = b >= BS - 2

                def cutout(ap_all, ap_iv):
                    owin = ap_all[..., bass.ds(cs, CW)]
                    cut_eng.tensor_tensor(owin, owin, ap_iv, Alu.mult)

                for c in range(C):
                    # out = A*x + D
                    nc.vector.scalar_tensor_tensor(
                        out=o_t[:, c],
                        in0=x_t[:, c],
                        scalar=pf_sb[:, b, I_A : I_A + 1],
                        in1=m3_t[:],
                        op0=Alu.mult,
                        op1=Alu.add,
                    )
                    if perch:
                        cutout(o_t[:, c], invw_sb[:, b])
                        dst_base = out[b, c]
                        dst = bass.AP(
                            tensor=dst_base.tensor,
                            offset=dst_base.offset,
                            ap=[[NJ * W, P], [W, NJ], [1, W]],
                        )
                        nc.gpsimd.dma_start(out=dst, in_=o_t[:, c])
                if not perch:
                    # one multiply of the CW-wide window at dynamic start cs
                    # across all channels (one register slot)
                    cutout(
                        o_t[:],
                        invw_sb[:, b : b + 1].broadcast_to([P, C, NJ, CW]),
                    )
                    for c in range(C):
                        dst_base = out[b, c]
                        dst = bass.AP(
                            tensor=dst_base.tensor,
                            offset=dst_base.offset,
                            ap=[[NJ * W, P], [W, NJ], [1, W]],
                        )
                        nc.gpsimd.dma_start(out=dst, in_=o_t[:, c])

            # software-pipelined emission: out(b-2) first so gpsimd's store
            # issue is never queued behind newer waits, then load(b), m3(b-1)
            for i in range(BS + 2):
                if 0 <= i - 2 < BS:
                    stage_out(i - 2)
                if i < BS:
                    stage_load(i)
                if 0 <= i - 1 < BS:
                    stage_m3(i - 1)

    _split_waits(nc)
    return nc


_cache = threading.local()


def _get_program():
    nc = getattr(_cache, "nc", None)
    if nc is None:
        nc = _build_program()
        _cache.nc = nc
    return nc


def _host_params(images, rand01):
    """Per-sample parameters, computed with float32 semantics matching the
    jax reference."""
    r = np.asarray(rand01, dtype=np.float32).reshape(7, B)
    th = np.floor(r[0] * np.float32(2 * PAD + 1)).astype(np.int32) - PAD
    tw = np.floor(r[1] * np.float32(2 * PAD + 1)).astype(np.int32) - PAD
    badd = r[2] - np.float32(0.5)
    s = r[3] * np.float32(2.0)
    t = r[4] + np.float32(0.5)
    ch = round(H * 0.2)  # 102
    cw = round(W * 0.2)
    oh = np.floor(r[5] * np.float32(H + (1 - ch % 2))).astype(np.int32)
    ow = np.floor(r[6] * np.float32(W + (1 - cw % 2))).astype(np.int32)

    A = t * s
    BC = t * (np.float32(1.0) - s) / np.float32(3.0)
    GS = (np.float32(1.0) - t) / np.float32(3 * H * W)
    pf = np.stack([A, BC, GS, badd], axis=1).astype(np.float32)  # [B,4]
    # fused element offset of the flat-shifted window within img[b, c]
    pi = ((th + PAD + 1).astype(np.int64) * W + tw).astype(np.int32)[
        :, None
    ]  # [B,1]

    # edge masks for the column spill: head cols [0,64) die when tw<0
    # (col < -tw), tail cols [448,512) die when tw>0 (col >= 512-tw)
    k = np.arange(PAD)
    el = np.ones((B, 2, PAD), dtype=np.float32)
    el[:, 0, :] = (k[None, :] >= -tw[:, None]).astype(np.float32)
    el[:, 1, :] = ((W - PAD + k)[None, :] < (W - tw)[:, None]).astype(np.float32)

    idx = np.arange(H)
    a0 = np.maximum(0, oh - ch // 2)[:, None]
    a1 = np.minimum(H - 1, oh + (ch - ch // 2) - 1)[:, None]
    b0 = np.maximum(0, ow - cw // 2)[:, None]
    b1 = np.minimum(W - 1, ow + (cw - cw // 2) - 1)[:, None]
    rowz = (idx[None, :] >= a0) & (idx[None, :] <= a1)  # [B,H]
    colz = (idx[None, :] >= b0) & (idx[None, :] <= b1)  # [B,W]
    pcs = np.minimum(b0[:, 0], W - CW).astype(np.int32)[:, None]  # [B,1]
    # inverse cutout mask on the CW-wide window starting at pcs, packed
    # partition-major: row r = 4p + j
    wi = pcs + np.arange(CW)[None, :]  # [B,CW]
    colz_win = np.take_along_axis(colz, wi, axis=1)  # [B,CW]
    invw = (
        1.0 - rowz[:, :, None] * colz_win[:, None, :]
    ).astype(ml_dtypes.bfloat16)  # [B,H,CW]
    invw = invw.reshape(B, P, NJ, CW)  # row r=(p j) -> [B,P,NJ,CW]

    imp = np.zeros((B, C, HV, W), dtype=np.float32)
    imp[:, :, PAD + 1 : PAD + 1 + H, :] = images
    return imp, pf, pi, pcs, el, invw


def _run(images, rand01, trace=False):
    images = np.ascontiguousarray(np.asarray(images, dtype=np.float32))
    imp, pf, pi, pcs, el, invw = _host_params(images, rand01)
    nc = _get_program()
    in_maps = [
        {
            "img": np.ascontiguousarray(imp[k * BS : (k + 1) * BS]),
            "pf": np.ascontiguousarray(pf[k * BS : (k + 1) * BS]),
            "pi": np.ascontiguousarray(pi[k * BS : (k + 1) * BS]),
            "pcs": np.ascontiguousarray(pcs[k * BS : (k + 1) * BS]),
            "el": np.ascontiguousarray(el[k * BS : (k + 1) * BS]),
            "invw": np.ascontiguousarray(invw[k * BS : (k + 1) * BS]),
        }
        for k in range(M)
    ]
    res = run_bass_kernel_spmd(nc, in_maps, list(range(M)), trace=trace)
    full = np.concatenate([res.results[k]["out"] for k in range(M)], axis=0)
    return full, res


def kernel(images, rand01):
    full, _ = _run(images, rand01, trace=False)
    return full
